# revision 41
# baseline (speedup 1.0000x reference)
"""Cross-attention kernel for 8 Trainium2 NeuronCores (SPMD).

Problem: B=4, T_q=T_kv=2048, Q_DIM=1024, KV_DIM=768, H=16, DK=64, fp32.
  q = q_tokens @ Wq.T ; k = kv_tokens @ Wk.T ; v = kv_tokens @ Wv.T
  out = softmax(q k^T / sqrt(DK)) v @ Wo.T

Sharding (8 cores): core c handles batch b=c//2 and head-group hg=c%2
(8 heads, 512 of the 1024 q-dims).  After attention, the pair (2b, 2b+1)
AllGathers the per-head-group attention outputs, then each core runs the
output projection against ITS half of the Wo columns — core c returns
out[b, :, (c%2)*512:(c%2+1)*512] transposed.  The rank-dependent
output-channel split lives entirely in the host-side Wo slice, so the
device program is identical on all cores.

This version is organized around the scalar engine being the hard
bottleneck: softmax exp is 33.5M elements/core = ~285us of ACT time
that nothing else can absorb.  Everything else is scheduled to hide
under it:
  - bf16 everywhere on device (fp32 PSUM accumulation).  Halves DMA /
    SBUF / collective bytes and makes weight loads cheap, so K/Q/V stay
    SBUF-resident (no DRAM round-trip).
  - A minimal prelude (K+first-Qs for head-pair 0, half of V) lets exp
    start ~20us in; all remaining projection matmuls are drip-fed into
    the attention loop's PE slack via a deferred-work queue of
    generators (one yield per matmul), ordered by need-by time.
  - Softmax runs without max-subtraction (scores are O(6)); the
    denominator comes from an appended ones-column in V during the PV
    matmul; normalization uses the single-op reciprocal_approx_fast.
  - Head-pairs 0-2 AllGather their attention output once each
    (overlapped with the next head-pair's compute); head-pair 3
    exchanges per 512-token j-block, and the output projection is
    drip-fed into head-pair 3's attention loops the same way the
    input projections were.
"""

import numpy as np

import concourse.bacc as bacc
import concourse.mybir as mybir
import concourse.tile as tile
from concourse import bass_utils

try:
    import ml_dtypes
    _BF16 = ml_dtypes.bfloat16
except ImportError:  # pragma: no cover
    _BF16 = mybir.dt.np(mybir.dt.bfloat16)

N_CORES = 8
P = 128
TQ = 2048
TKV = 2048
CQ = 1024     # q_tokens channels
CKV = 768     # kv_tokens channels
DQ = 512      # per-core head-group q dims (8 heads x 64)
DO = 512      # per-core output channels (half of 1024)
NJ = 4        # 512-wide tq j-blocks (== projection t-blocks)
NI = TKV // P  # 16 kv chunks
NHP = DQ // P  # 4 head-pairs
CQ_CH = CQ // P   # 8
CKV_CH = CKV // P  # 6
NCC = 2 * NHP     # 8 dc chunks in the gathered attention output

F32 = mybir.dt.float32
BF = mybir.dt.bfloat16
EXP = mybir.ActivationFunctionType.Exp
MUL = mybir.AluOpType.mult

_compiled = None


def _build():
    nc = bacc.Bacc("TRN2", target_bir_lowering=False, debug=False,
                   num_devices=N_CORES)

    xqT = nc.dram_tensor("xqT", [CQ, TQ], BF, kind="ExternalInput")
    xkvT = nc.dram_tensor("xkvT", [CKV, TKV], BF, kind="ExternalInput")
    # weights come pre-chunked from the host as [128, n_chunks*512] so the
    # resident loads are fully contiguous (a strided gather here costs
    # >20us of head latency)
    wqT = nc.dram_tensor("wqT", [P, CQ_CH * DQ], BF, kind="ExternalInput")
    wkT = nc.dram_tensor("wkT", [P, CKV_CH * DQ], BF, kind="ExternalInput")
    wvT = nc.dram_tensor("wvT", [P, CKV_CH * DQ], BF, kind="ExternalInput")
    # full-dc Wo slice for this core's output-channel half, dc rows in
    # gathered order (head-group 0 rows then head-group 1 rows)
    woT = nc.dram_tensor("woT", [P, NCC * DO], BF, kind="ExternalInput")
    onesc = nc.dram_tensor("onesc", [P, 8], BF, kind="ExternalInput")
    out_ext = nc.dram_tensor("out", [DO, TQ], F32, kind="ExternalOutput")

    groups = [[2 * b, 2 * b + 1] for b in range(N_CORES // 2)]

    with tile.TileContext(nc) as tc:
        with (
            tc.tile_pool(name="weights", bufs=1) as wpool,
            tc.tile_pool(name="xres", bufs=1) as xpool,
            tc.tile_pool(name="kqv", bufs=1) as kpool,
            tc.tile_pool(name="stage", bufs=1) as stpool,
            tc.tile_pool(name="dram", bufs=1, space="DRAM") as dpool,
            tc.tile_pool(name="psum_s", bufs=2, space="PSUM") as ps_s,
            tc.tile_pool(name="psum_pv", bufs=2, space="PSUM") as ps_pv,
            tc.tile_pool(name="psum_pj", bufs=2, space="PSUM") as ps_pj,
        ):
            # ---- resident weights + token inputs (bf16) ----
            wk_sb = wpool.tile([P, CKV_CH, DQ], BF, tag="wk")
            wq_sb = wpool.tile([P, CQ_CH, DQ], BF, tag="wq")
            wv_sb = wpool.tile([P, CKV_CH, DQ], BF, tag="wv")
            wo_sb = wpool.tile([P, NCC, DO], BF, tag="wo")
            ones_sb = wpool.tile([P, 8, 1], BF, tag="ones")
            # Resident loads, split per 128KB chunk and spread over the
            # three DMA-capable queues (sync/SP, gpsimd, scalar/ACT — the
            # latter idle until the first exp).  DMA transfers fair-share
            # HBM bandwidth, so the issue ORDER is the priority order:
            # wk+xkv chunks feed the prelude K matmuls within ~10us,
            # wv feeds V, wq+xq(tb0) gate Q(0,0) and the first exp;
            # xq(tb1-3) and wo stream in during attention.
            xkv_sb = [xpool.tile([P, TKV], BF, tag="xkv", bufs=CKV_CH,
                                 name=f"xkv{c}") for c in range(CKV_CH)]
            xq_sb = [xpool.tile([P, TQ], BF, tag="xq", bufs=CQ_CH,
                                name=f"xq{c}") for c in range(CQ_CH)]
            # aggregate input-load bandwidth is ~150GB/s, so the ISSUE
            # order is the arrival order.  Everything arrives per-128KB
            # chunk, tb-block-major, so K(0,0)+Q(0,0) (the first-exp gate)
            # wait on only ~3.5MB; later tb blocks stream in just ahead
            # of their consumers (K(0,tb) pops, V chunks, Q blocks).
            q3 = [nc.sync, nc.gpsimd, nc.scalar]
            # need-time order at ~250GB/s aggregate: the first exp is
            # gated by wk+xkv(tb0)+wq+xq(tb0)+wv (~4.25MB), so those ship
            # first; the xkv/xq remainders follow, arriving just ahead of
            # the K(0,tb)/V-chunk/Q(0,1) pops inside j0's loop.
            for c in range(CKV_CH):
                q3[c % 3].dma_start(wk_sb[:, c, :],
                                    wkT.ap()[:, c * DQ:(c + 1) * DQ])
                q3[c % 3].dma_start(xkv_sb[c][:, 0:512],
                                    xkvT.ap()[c * P:(c + 1) * P, 0:512])
            for c in range(CQ_CH):
                q3[c % 3].dma_start(wq_sb[:, c, :],
                                    wqT.ap()[:, c * DQ:(c + 1) * DQ])
                q3[c % 3].dma_start(xq_sb[c][:, 0:512],
                                    xqT.ap()[c * P:(c + 1) * P, 0:512])
            for c in range(CKV_CH):
                q3[c % 3].dma_start(wv_sb[:, c, :],
                                    wvT.ap()[:, c * DQ:(c + 1) * DQ])
            nc.gpsimd.dma_start(ones_sb[:],
                                onesc.ap().rearrange("p (n o) -> p n o", o=1))
            for c in range(CKV_CH):
                q3[c % 3].dma_start(xkv_sb[c][:, 512:TKV],
                                    xkvT.ap()[c * P:(c + 1) * P, 512:TKV])
            for c in range(CQ_CH):
                q3[c % 3].dma_start(xq_sb[c][:, 512:TQ],
                                    xqT.ap()[c * P:(c + 1) * P, 512:TQ])
            for cc in range(NCC):
                nc.scalar.dma_start(wo_sb[:, cc, :],
                                    woT.ap()[:, cc * DO:(cc + 1) * DO])

            # ---- SBUF-resident K/Q/V (written by projection evictions) ----
            # kb[hp][tb]: [128 dk (2 heads), 512 tkv]; qs[hp][tb] same for tq
            kb = [[kpool.tile([P, 512], BF, tag="kb", bufs=NHP * NJ,
                              name=f"kb{hp}_{tb}") for tb in range(NJ)]
                  for hp in range(NHP)]
            qs = [[kpool.tile([P, 512], BF, tag="qs", bufs=NHP * NJ,
                              name=f"qs{hp}_{tb}") for tb in range(NJ)]
                  for hp in range(NHP)]
            # vt[tc]: [128 tkv-chunk, 8 heads, 64+1] (ones col -> denominator)
            vt = [kpool.tile([P, 8, 65], BF, tag="vt", bufs=NI,
                             name=f"vt{tc}") for tc in range(NI)]
            # normalized attention output per head-pair (exchanged via CC)
            ao = [kpool.tile([P, TQ], BF, tag="ao", bufs=NHP,
                             name=f"ao{hp}") for hp in range(NHP)]

            # ---- internal DRAM for collectives ----
            agi = [dpool.tile([P, TQ], BF, tag=f"agi{h}", name=f"agi{h}")
                   for h in range(NHP - 1)]
            ago = [dpool.tile([2, P, TQ], BF, tag=f"ago{h}", name=f"ago{h}")
                   for h in range(NHP - 1)]
            agi3 = [dpool.tile([P, 512], BF, tag=f"agi3_{j}", name=f"agi3_{j}")
                    for j in range(NJ)]
            ago3 = [dpool.tile([2, P, 512], BF, tag=f"ago3_{j}",
                               name=f"ago3_{j}") for j in range(NJ)]

            # ============ projection work units (one yield per MM) ========
            def k_gen(hp, tb):
                pk = ps_pj.tile([P, 512], F32, tag="pj", name=f"pk_{hp}_{tb}")
                for c in range(CKV_CH):
                    nc.tensor.matmul(pk[:], wk_sb[:, c, hp * P:(hp + 1) * P],
                                     xkv_sb[c][:, tb * 512:(tb + 1) * 512],
                                     start=(c == 0), stop=(c == CKV_CH - 1))
                    if c == CKV_CH - 1:
                        nc.vector.tensor_copy(kb[hp][tb][:], pk[:])
                    yield

            def q_gen(hp, tb):
                pq = ps_pj.tile([P, 512], F32, tag="pj", name=f"pq_{hp}_{tb}")
                for c in range(CQ_CH):
                    nc.tensor.matmul(pq[:], wq_sb[:, c, hp * P:(hp + 1) * P],
                                     xq_sb[c][:, tb * 512:(tb + 1) * 512],
                                     start=(c == 0), stop=(c == CQ_CH - 1))
                    if c == CQ_CH - 1:
                        nc.vector.tensor_copy(qs[hp][tb][:], pq[:])
                    yield

            def v_gen(tc_i):
                pv = ps_pj.tile([P, 512], F32, tag="pj", name=f"pv_{tc_i}")
                for c in range(CKV_CH):
                    nc.tensor.matmul(
                        pv[:],
                        xkv_sb[c][:, tc_i * P:(tc_i + 1) * P],
                        wv_sb[:, c, :],
                        start=(c == 0), stop=(c == CKV_CH - 1))
                    if c == CKV_CH - 1:
                        nc.vector.tensor_copy(
                            vt[tc_i][:, :, 0:64],
                            pv[:].rearrange("p (h d) -> p h d", d=64))
                        nc.vector.tensor_copy(vt[tc_i][:, :, 64:65],
                                              ones_sb[:])
                    yield

            def run_all(gen):
                for _ in gen:
                    pass

            # deferred projection work, drip-fed into the attention loop.
            # Order respects need-by times: Q(hp,tb) before block (hp,tb)
            # starts, K(hp) fully before head-pair hp starts.
            deferred = [
                q_gen(0, 2),
                k_gen(1, 0), k_gen(1, 1),
                q_gen(0, 3),
                k_gen(1, 2), k_gen(1, 3),
                q_gen(1, 0), q_gen(1, 1),
                k_gen(2, 0), k_gen(2, 1), k_gen(2, 2), k_gen(2, 3),
                q_gen(1, 2), q_gen(1, 3),
                q_gen(2, 0),
                k_gen(3, 0), k_gen(3, 1),
                q_gen(2, 1), q_gen(2, 2),
                k_gen(3, 2), k_gen(3, 3),
                q_gen(2, 3),
                q_gen(3, 0), q_gen(3, 1), q_gen(3, 2), q_gen(3, 3),
            ]
            deferred.reverse()

            def pop_work(queue, n):
                while n > 0 and queue:
                    gen = queue[-1]
                    try:
                        next(gen)
                        n -= 1
                    except StopIteration:
                        queue.pop()

            # ================= prelude =================
            # K(0,0) + Q(0,0) only — the minimum for the exp stream to
            # start.  ALL of V and K(0,1..3) are produced inside j0's
            # iterations as their tb-blocks arrive from DRAM: exp needs
            # only the scores, and the PV consumer can lag behind the
            # exp stream on the deep ex ring.  Q(0,1) is drip-fed into
            # j0's later iterations.
            run_all(k_gen(0, 0))
            run_all(q_gen(0, 0))
            k0 = {1: [k_gen(0, 1)], 2: [k_gen(0, 2)], 5: [k_gen(0, 3)]}
            q01 = [q_gen(0, 1)]

            # ============== out-projection work units ==============
            # aog/ost DMAs stay off the gpsimd queue so the normalize
            # broadcasts never wait behind them; the tail block (j=3) can
            # use the scalar queue, idle once the last exp has issued.
            def po_gen(j):
                js = slice(j * 512, (j + 1) * 512)
                rhs = []
                for n in range(NCC):
                    g, hpx = n % 2, n // 2
                    aog = stpool.tile([P, 512], BF, tag="aog", bufs=9,
                                      name=f"aog_{j}_{g}_{hpx}")
                    eng = nc.scalar if (j == NJ - 1 and n % 2) else nc.sync
                    if hpx < NHP - 1:
                        eng.dma_start(aog[:], ago[hpx][g, :, js])
                    else:
                        eng.dma_start(aog[:], ago3[j][g, :, :])
                    rhs.append(aog)
                for do in range(DO // P):
                    po = ps_pj.tile([P, 512], F32, tag="pj",
                                    name=f"po_{j}_{do}")
                    for n in range(NCC):
                        cc = (n % 2) * NHP + n // 2
                        nc.tensor.matmul(
                            po[:], wo_sb[:, cc, do * P:(do + 1) * P],
                            rhs[n][:],
                            start=(n == 0), stop=(n == NCC - 1))
                        yield
                    ost = stpool.tile([P, 512], F32, tag="ost", bufs=3,
                                      name=f"ost_{j}_{do}")
                    nc.vector.tensor_copy(ost[:], po[:])
                    oeng = nc.scalar if j == NJ - 1 else nc.sync
                    oeng.dma_start(out_ext[do * P:(do + 1) * P, js],
                                   ost[:])

            po_work = []

            # ================= attention =================
            for hp in range(NHP):
                for j in range(NJ):
                    js = slice(j * 512, (j + 1) * 512)
                    acc_a = ps_pv.tile([P, 512], F32, tag="pv")
                    acc_b = ps_pv.tile([P, 512], F32, tag="pv")
                    for i in range(NI):
                        tbk, ik = i // 4, i % 4
                        ks = slice(ik * 128, (ik + 1) * 128)
                        sc = ps_s.tile([P, 1024], F32, tag="sc")
                        nc.tensor.matmul(sc[:, 0:512],
                                         kb[hp][tbk][0:64, ks],
                                         qs[hp][j][0:64, :],
                                         start=True, stop=True)
                        nc.tensor.matmul(sc[:, 512:1024],
                                         kb[hp][tbk][64:128, ks],
                                         qs[hp][j][64:128, :],
                                         start=True, stop=True)
                        ex = stpool.tile([P, 1024], BF, tag="ex", bufs=6)
                        nc.scalar.activation(ex[:], sc[:], EXP, scale=0.125)
                        # first block: K(0,tb) and V chunk i are produced
                        # inline as their tb-blocks land from DRAM; Q(0,1)
                        # lands in the later iterations, in time for j1.
                        if hp == 0 and j == 0:
                            if i in k0:
                                run_all(k0[i][0])
                            run_all(v_gen(i))
                            if i >= 2:
                                pop_work(q01, 2)
                        nc.tensor.matmul(acc_a[0:65, :],
                                         vt[i][:, 2 * hp, :],
                                         ex[:, 0:512],
                                         start=(i == 0), stop=(i == NI - 1))
                        nc.tensor.matmul(acc_b[0:65, :],
                                         vt[i][:, 2 * hp + 1, :],
                                         ex[:, 512:1024],
                                         start=(i == 0), stop=(i == NI - 1))
                        if not (hp == 0 and j == 0):
                            pop_work(deferred, 2 if j == 3 else 1)
                        # drip-feed the out projection into head-pair 3;
                        # nothing pops during j3 — po(2) is not gated by
                        # the final collective, so it runs in the tail
                        # during that otherwise-idle exchange wait.
                        if hp == NHP - 1 and j in (1, 2):
                            pop_work(po_work, 3)
                    # evict BOTH accumulators first (frees the PSUM ring for
                    # the next j-block before the slow reciprocals run),
                    # then normalize: ao[:, js] = acc[0:64] / acc[64]
                    pvsts, recs, bcs = [], [], []
                    for half, acc in ((0, acc_a), (1, acc_b)):
                        pvst = stpool.tile([P, 512], F32, tag="pvst", bufs=4,
                                           name=f"pvst_{hp}_{j}_{half}")
                        nc.vector.tensor_copy(pvst[0:65, :], acc[0:65, :])
                        pvsts.append(pvst)
                    for half in (0, 1):
                        # reciprocal_approx_fast is only correct with
                        # partition-0 operands (verified on hw), so hop the
                        # denominator row down first
                        den = stpool.tile([P, 512], F32, tag="den", bufs=2)
                        nc.vector.tensor_copy(den[0:1, :],
                                              pvsts[half][64:65, :])
                        rec = stpool.tile([P, 512], F32, tag="rec", bufs=2)
                        nc.vector.reciprocal_approx_fast(rec[0:1, :],
                                                         den[0:1, :])
                        recs.append(rec)
                        bc = stpool.tile([P, 512], F32, tag="bc", bufs=2)
                        nc.gpsimd.partition_broadcast(bc[0:64, :],
                                                      rec[0:1, :],
                                                      channels=64)
                        bcs.append(bc)
                    for half in (0, 1):
                        nc.vector.tensor_tensor(
                            ao[hp][half * 64:(half + 1) * 64, js],
                            pvsts[half][0:64, :], bcs[half][0:64, :], op=MUL)
                    # exchanges: head-pairs 0-2 once per hp (overlapped with
                    # the next head-pair); head-pair 3 per j-block so the
                    # output projection can start before attention ends.
                    if hp == NHP - 1:
                        nc.sync.dma_start(agi3[j][:], ao[hp][:, js])
                        nc.gpsimd.collective_compute(
                            "AllGather", mybir.AluOpType.bypass,
                            replica_groups=groups,
                            ins=[agi3[j].opt()], outs=[ago3[j].opt()])
                        if j < NJ - 1:
                            po_work.insert(0, po_gen(j))
                if hp < NHP - 1:
                    nc.sync.dma_start(agi[hp][:], ao[hp][:])
                    nc.gpsimd.collective_compute(
                        "AllGather", mybir.AluOpType.bypass,
                        replica_groups=groups,
                        ins=[agi[hp].opt()], outs=[ago[hp].opt()])

            # ===== output projection tail =====
            for gen in reversed(po_work):
                run_all(gen)
            run_all(po_gen(NJ - 1))

    nc.compile()
    return nc


def make_in_maps(q_tokens, kv_tokens, Wq, Wk, Wv, Wo):
    q_tokens = np.asarray(q_tokens, np.float32)
    kv_tokens = np.asarray(kv_tokens, np.float32)
    Wq = np.asarray(Wq, np.float32)
    Wk = np.asarray(Wk, np.float32)
    Wv = np.asarray(Wv, np.float32)
    Wo = np.asarray(Wo, np.float32)
    def chunked(w):
        # [in, out] -> [128, n_chunks*out]: contiguous per-partition image
        # of the SBUF-resident [P, n, out] weight tiles
        n = w.shape[0] // P
        return np.ascontiguousarray(
            w.reshape(n, P, w.shape[1]).transpose(1, 0, 2).reshape(P, -1)
        ).astype(_BF16)

    in_maps = []
    for c in range(N_CORES):
        b, hg = c // 2, c % 2
        sl = slice(hg * DQ, (hg + 1) * DQ)
        osl = slice(hg * DO, (hg + 1) * DO)
        in_maps.append({
            "xqT": np.ascontiguousarray(q_tokens[b].T).astype(_BF16),
            "xkvT": np.ascontiguousarray(kv_tokens[b].T).astype(_BF16),
            "wqT": chunked(Wq[sl, :].T),
            "wkT": chunked(Wk[sl, :].T),
            "wvT": chunked(Wv[sl, :].T),
            # [dc, do-half] with dc rows in gathered (global head) order
            "woT": chunked(Wo[osl, :].T),
            "onesc": np.ones((P, 8), _BF16),
        })
    return in_maps


def kernel(q_tokens, kv_tokens, Wq, Wk, Wv, Wo):
    global _compiled
    if _compiled is None:
        _compiled = _build()
    nc = _compiled

    in_maps = make_in_maps(q_tokens, kv_tokens, Wq, Wk, Wv, Wo)
    res = bass_utils.run_bass_kernel_spmd(nc, in_maps,
                                          core_ids=list(range(N_CORES)))
    B = 4
    out = np.empty((B, TQ, 2 * DO), np.float32)
    for c in range(N_CORES):
        b, hg = c // 2, c % 2
        out[b, :, hg * DO:(hg + 1) * DO] = res.results[c]["out"].T
    return out
